# revision 42
# baseline (speedup 1.0000x reference)
"""Multi-head attention (B=4, S=2048, D=768, H=12) on 8 TRN2 NeuronCores.

Sharding: core i handles batch b = i//2 and head-group g = i%2 (6 heads of 64).
Each core computes Q/K/V projections for its head slice, attention, and a
partial output projection (row-slice of Wo). Host sums the two partials per
batch and adds bo.

Device layout choices:
  - x is fed pre-transposed as xT [D, S] so all projection matmuls contract
    over D on the partition dim.
  - Q, K are produced transposed: QT/KT [384, S] (head dim on partitions).
  - logits are computed transposed, logitsT [k, q]: lhsT = KT_h [64, k-tile],
    rhs = QT_h [64, q-tile]. The additive mask (per-k) then lands on the
    partition dim, so it rides the exp() activation's per-partition bias.
    Two consecutive slots' logits (4 K=64 mms) are emitted back-to-back
    (PAIR_KC): longer uniform K=64 runs lower the per-mm cost vs
    alternating with full-array mms (HW-measured ~16 ns/mm).
  - Softmax skips max-subtraction (logits are O(5), exp is safe in fp32);
    masked positions get bias -1e9 -> exp == 0.
  - V is kept in natural [k, c] layout, augmented with a ones column, so the
    PV matmul (lhsT = V'_h [k-tile, 65], rhs = probsT [k-tile, q-tile])
    accumulates both ctxT [64, q] and the softmax denominator (row 64) in one
    accumulation group.
  - Normalization: the denominator row is copied to partitions 0 and 32
    of a static once-memset staging tile, reciprocal_approx_fast'd in
    place ([0:33]: custom-DVE ops need base partition 0, and every lane
    must see defined fp32 data — garbage lanes NaN'd whole first-exec
    outputs), then broadcast across all 64 partitions with one DVE
    stream_shuffle (mask=[0]*32 replicates each 32-partition quadrant's
    row 0) — all-DVE, no DMA (SHUF_BC); the normalized pair lands
    stacked in one [128, 512] tile.
  - Output projection contracts over the stacked head-pair dim: lhsT =
    ctx_pair [128, q-tile], rhs = Wo_pair [128, e-tile], K=128 full array,
    accumulating 3 pairs into one PSUM tile; result is in natural [q, e]
    layout for the store.
  - Scheduling: the softmax exp stream on the scalar engine is the binding
    resource (~200us/core: 6 heads x 2048^2 logits / 128 lanes / 1.2GHz,
    exp runs nowhere else), so the whole kernel is a flat stream of 192
    (pair, kc) slots paced by one [128, 1024] exp per slot. Each slot emits
    logits+exp first, then one fill-work unit, then a cross-pair-lagged PV,
    then at most one deferred output-projection group. The fill map spreads
    the V builds and QT/KT projection chunks across the stream using their
    exact read windows (qt chunk cc's sc-slice is read only by pair
    (sc, cc); kt cc is locked until (3, cc)), rebuilding most chunks for
    the NEXT loop iteration right after their last reader so an iteration
    opens straight into logits. kt1 rebuilds in the drain bridge the loop
    boundary so the PE HAM clock gate stays at full rate.
  - The softmax normalization chain (raw-copy -> reciprocal ->
    stream_shuffle broadcast -> multiply) runs off the critical path: a
    single [65, 512] bf16 copy per head releases the PSUM accumulators
    ~1.3us after the last PV. The earlier rec->dram->readback DMA pair was
    an untracked-DRAM ordering hazard: emission-order changes raced it and
    corrupted single-shot runs while reps loops masked the corruption
    (identical inputs -> stale reads look correct). SHUF_BC removes the
    hazard class entirely, which is what makes PAIR_KC and HW_DMA safe.
  - V bias rides a DVE broadcast add (VB_DVE) instead of a rank-1 matmul,
    keeping the V build to 6 full-K matmuls (measured -8us).
  - All matmul operands are bf16 (full PE speed; fp32 PSUM accumulate).
  - Measured on HW (8-core SPMD, per-iteration of a reps loop):
    baseline 302us -> 282-285us depending on ambient device state (the
    device's effective clock drifts ~5% between sessions, especially
    after NRT faults; judge A/B tests back-to-back only). Engine budget:
    PE ~255us busy (bottleneck), ACT exp stream ~209us (192 x ~1090ns),
    DVE ~129us, Pool/gpsimd near idle. Single-shot (reps=1) correctness
    re-verified 3x+ after every emission-order change. LAG=7 hard-faults
    the device (NRT_EXEC_UNIT_UNRECOVERABLE, 2/2 runs — suspected PE
    wait-queue overflow from too many simultaneously-blocked PVs);
    LAG=6/LAG0=7 is the deepest safe point and the LAG0=7 retune is
    worth ~2us.
"""

import numpy as np
from contextlib import ExitStack

S = 2048
D = 768
HL = 6  # heads per core
HD = 64
CPB = 384  # channels per core = HL * HD
DC = D // 128  # 6 contraction chunks
CC = CPB // 128  # 3 chunks of QT/KT partitions
NQ4 = S // 512  # 4 q chunks of 512
NK = S // 128  # 16 k chunks of 128
NEG_BIG = -1.0e9
LAG = 6   # PV trails logits by this many slots (crossing pair boundaries)
LAG0 = 7  # deeper lag on the first NP0 pairs, widening the V-build windows
          # (re-tuned +1 after PAIR_KC shifted fills ~1 slot later; -2us)
NP0 = 2   # pairs running at LAG0 before dropping to LAG
PAD_LOGITS = False  # full-K=128 zero-padded logits mms: measured +18us vs
                    # the row-tiled K=64 pairs in-context — keep False
VB_DVE = True       # V bias via DVE broadcast add (vs rank-1 ones matmul)
PAIR_KC = True      # emit two consecutive slots' logits back-to-back
                    # (safe only with SHUF_BC: with the DMA-based rec
                    # broadcast, emission-order shifts raced the rec_dram
                    # readback and corrupted single-shot runs)
SHUF_BC = True      # softmax-reciprocal broadcast via DVE stream_shuffle
                    # (den copied to partitions 0+32, mask=[0]*32) instead
                    # of the rec->dram->broadcast DMA pair: removes the only
                    # untracked-DRAM ordering hazard and 48 SWDGE DMAs
PV_FIRST = False    # within a slot: PV pops before fill work
HW_DMA = False      # out stores on HWDGE (sync) queues. Re-tried after the
                    # uninit-dd root cause was fixed: correct 4/4 fresh-process
                    # but no longer faster (285.4 vs 285.1us back-to-back) —
                    # its earlier -2us came from relieving SWDGE congestion
                    # that SHUF_BC has since eliminated. Keep the longer
                    # track record (stores on the strictly-FIFO SWDGE queue)
MERGE_REST = False  # merge both paired slots' fills+pops into one run

_cache = {}


def _build_nc(reps=1, parts="all"):
    import concourse.bass as bass
    import concourse.mybir as mybir
    import concourse.tile as tile
    from concourse import bacc
    from contextlib import nullcontext

    f32 = mybir.dt.float32
    bf16 = mybir.dt.bfloat16
    AF = mybir.ActivationFunctionType

    nc = bacc.Bacc("TRN2", target_bir_lowering=False, debug=False,
                   enable_asserts=False)

    xt = nc.dram_tensor("xt", [D, S], bf16, kind="ExternalInput").ap()
    wq = nc.dram_tensor("wq", [D, CPB], bf16, kind="ExternalInput").ap()
    wk = nc.dram_tensor("wk", [D, CPB], bf16, kind="ExternalInput").ap()
    wv = nc.dram_tensor("wv", [D, CPB], bf16, kind="ExternalInput").ap()
    wo = nc.dram_tensor("wo", [CPB, D], bf16, kind="ExternalInput").ap()
    bqk = nc.dram_tensor("bqk", [128, 2 * CC], f32, kind="ExternalInput").ap()
    bv = nc.dram_tensor("bv", [128, CPB], bf16, kind="ExternalInput").ap()
    maskb = nc.dram_tensor("maskb", [128, NK], f32, kind="ExternalInput").ap()
    out = nc.dram_tensor("out", [S, D], f32, kind="ExternalOutput").ap()
    rec_dram = nc.dram_tensor("rec_dram", [NQ4 * HL, 512], f32).ap()

    with tile.TileContext(nc) as tc, ExitStack() as top:
        const = top.enter_context(tc.tile_pool(name="const", bufs=1))

        # ---- constant loads ----
        wq_sb = const.tile([128, DC, CPB], bf16, tag="wq")
        wk_sb = const.tile([128, DC, CPB], bf16, tag="wk")
        wv_sb = const.tile([128, DC, CPB], bf16, tag="wv")
        for dc in range(DC):
            nc.sync.dma_start(out=wq_sb[:, dc, :], in_=wq[dc * 128:(dc + 1) * 128, :])
            nc.sync.dma_start(out=wk_sb[:, dc, :], in_=wk[dc * 128:(dc + 1) * 128, :])
            nc.sync.dma_start(out=wv_sb[:, dc, :], in_=wv[dc * 128:(dc + 1) * 128, :])
        wo_sb = [const.tile([128, D], bf16, tag=f"wo{p}", name=f"wo_sb{p}")
                 for p in range(CC)]
        for p in range(CC):
            nc.sync.dma_start(out=wo_sb[p], in_=wo[p * 128:(p + 1) * 128, :])
        bqk_sb = const.tile([128, 2 * CC], f32, tag="bqk")
        nc.sync.dma_start(out=bqk_sb, in_=bqk)
        bv_bc = const.tile([128, CPB], bf16, tag="bvbc")
        nc.sync.dma_start(out=bv_bc, in_=bv)
        maskb_sb = const.tile([128, NK], f32, tag="maskb")
        nc.sync.dma_start(out=maskb_sb, in_=maskb)
        if not VB_DVE:
            ones_sb = const.tile([1, 128], bf16, tag="ones")
            nc.vector.memset(ones_sb, 1.0)

        qt_sb = [const.tile([128, S], bf16, tag=f"qt{c}", name=f"qt_sb{c}") for c in range(CC)]
        # Padded-K logits: per pair, two [128, S] KT tiles. ktA holds the even
        # head's 64 KT rows in partitions 0-63 (64-127 stay zero); ktB holds
        # the odd head's in partitions 64-127 (0-63 zero). The logits matmul
        # is then a full K=128 untiled matmul (background-weight-buffer
        # LDWEIGHTS pull-ahead, FWL) with the full stacked qt as rhs — the
        # zero rows cancel the other head's contribution. Measured on HW:
        # untiled K=128 mm ~194 ns vs the old row-tiled K=64 pair ~266 ns
        # each (no concurrency, un-hidden LDWEIGHTS).
        if PAD_LOGITS:
            ktA_sb = [const.tile([128, S], bf16, tag=f"ktA{c}",
                                 name=f"ktA_sb{c}") for c in range(CC)]
            ktB_sb = [const.tile([128, S], bf16, tag=f"ktB{c}",
                                 name=f"ktB_sb{c}") for c in range(CC)]
            for c in range(CC):
                nc.vector.memset(ktA_sb[c][64:128, :], 0.0)
                nc.vector.memset(ktB_sb[c][0:64, :], 0.0)
        else:
            kt_sb = [const.tile([128, S], bf16, tag=f"kt{c}",
                                name=f"kt_sb{c}") for c in range(CC)]
        v_sb = [const.tile([128, HL, HD + 1], bf16, tag=f"v{k}", name=f"v_sb{k}") for k in range(NK)]
        if SHUF_BC:
            # static dd staging tiles for the reciprocal-broadcast chain,
            # rows 1-31/33-63 memset once so the stream_shuffle never reads
            # uninitialized SBUF (first-exec NaN risk; later execs would be
            # silently masked by the previous run's residue)
            dd_tiles = [const.tile([HD, 512], f32, tag=f"dd{j}",
                                   name=f"dd_{j}") for j in range(4)]
            for j in range(4):
                nc.vector.memset(dd_tiles[j], 1.0)
            dd_ctr = [0]

        # xt tiles live in the never-closed const pool: reusing their SBUF
        # space would give later tile writers WAR/WAW waits on all 8 DMA
        # queues, exceeding HW sync-wait slots.
        xt_sb = [[const.tile([128, 512], bf16, tag=f"xt{dc}_{sc}",
                             name=f"xt_sb{dc}_{sc}") for sc in range(NQ4)]
                 for dc in range(DC)]

        # PSUM budget (8 banks): lg 2x2 + cps 2 + ops/mm shared 2 = 8
        lg_psum = top.enter_context(tc.tile_pool(name="lg", bufs=2, space="PSUM"))
        ctx_psum = top.enter_context(tc.tile_pool(name="cps", bufs=1, space="PSUM"))
        out_psum = top.enter_context(tc.tile_pool(name="ops", bufs=2, space="PSUM"))
        probs_pool = top.enter_context(tc.tile_pool(name="probs", bufs=12))
        rec_pool = top.enter_context(tc.tile_pool(name="rec", bufs=6))
        rbc_pool = top.enter_context(tc.tile_pool(name="rbc", bufs=4))
        raw_pool = top.enter_context(tc.tile_pool(name="raw", bufs=4))
        ctx_pool = top.enter_context(tc.tile_pool(name="ctx", bufs=2))
        outsb_pool = top.enter_context(tc.tile_pool(name="outsb", bufs=4))
        mm_psum = out_psum  # projection accumulators share the ops slots

        def chunk_group(cc, iw, sc):
            """One QT/KT projection group: 6 accumulating MMs + bias add.
            KT chunks land split across the zero-padded ktA/ktB tiles."""
            w_sb = (wq_sb, wk_sb)[iw]
            ps = mm_psum.tile([128, 512], f32, tag="ops",
                              name=f"qkps_{iw}_{cc}_{sc}")
            for dc in range(DC):
                nc.tensor.matmul(
                    ps,
                    lhsT=(w_sb[:, dc, cc * 128:(cc + 1) * 128]),
                    rhs=(xt_sb[dc][sc]),
                    start=(dc == 0), stop=(dc == DC - 1),
                )
            if iw == 0:
                nc.vector.tensor_scalar_add(
                    out=qt_sb[cc][:, sc * 512:(sc + 1) * 512], in0=ps,
                    scalar1=bqk_sb[:, cc:cc + 1],
                )
            elif PAD_LOGITS:
                sl = slice(sc * 512, (sc + 1) * 512)
                nc.vector.tensor_scalar_add(
                    out=ktA_sb[cc][0:64, sl], in0=ps[0:64, :],
                    scalar1=bqk_sb[0:64, CC + cc:CC + cc + 1],
                )
                nc.vector.tensor_scalar_add(
                    out=ktB_sb[cc][64:128, sl], in0=ps[64:128, :],
                    scalar1=bqk_sb[64:128, CC + cc:CC + cc + 1],
                )
            else:
                nc.vector.tensor_scalar_add(
                    out=kt_sb[cc][:, sc * 512:(sc + 1) * 512], in0=ps,
                    scalar1=bqk_sb[:, CC + cc:CC + cc + 1],
                )

        def v_group(kc):
            """V chunk kc, all 6 heads: natural [k, c] layout + ones column.
            bv is added on the DVE (broadcast tensor_tensor) so the PE group
            stays 6 full-K matmuls."""
            ps = mm_psum.tile([128, CPB], f32, tag="ops",
                              padded_shape=[128, 512], name=f"vps_{kc}")
            for dc in range(DC):
                nc.tensor.matmul(
                    ps,
                    lhsT=(xt_sb[dc][kc // 4][:, (kc % 4) * 128:
                                             (kc % 4 + 1) * 128]),
                    rhs=(wv_sb[:, dc, :]),
                    start=(dc == 0), stop=(VB_DVE and dc == DC - 1),
                )
            if VB_DVE:
                nc.vector.tensor_add(
                    v_sb[kc][:, :, 0:HD],
                    ps.rearrange("p (h d) -> p h d", h=HL),
                    bv_bc.rearrange("p (h d) -> p h d", h=HL),
                )
            else:
                nc.tensor.matmul(ps, lhsT=(ones_sb), rhs=(bv_bc[0:1, :]),
                                 start=False, stop=True)
                nc.vector.tensor_copy(
                    out=v_sb[kc][:, :, 0:HD],
                    in_=ps.rearrange("p (h d) -> p h d", h=HL),
                )
            nc.vector.memset(v_sb[kc][:, :, HD:HD + 1], 1.0)

        def wo_group(ctx_list, wqc, qs):
            """Output projection for q-subtile qs of q-chunk wqc: 3 K=128
            matmuls (stacked head pairs) accumulating into one PSUM tile."""
            ob = outsb_pool.tile([128, D], f32, tag="ob",
                                 name=f"ob_{wqc}_{qs}")
            for e0, en in ((0, 512), (512, 256)):
                ps = out_psum.tile([128, 512], f32, tag="ops",
                                   name=f"wops_{wqc}_{qs}_{e0}")
                for p in range(CC):
                    nc.tensor.matmul(
                        ps[:, 0:en],
                        lhsT=(ctx_list[p][:, qs * 128:(qs + 1) * 128]),
                        rhs=(wo_sb[p][:, e0:e0 + en]),
                        start=(p == 0), stop=(p == CC - 1),
                    )
                nc.vector.tensor_copy(out=ob[:, e0:e0 + en],
                                      in_=ps[:, 0:en])
            row = (wqc * 4 + qs) * 128
            dq = nc.sync if HW_DMA else nc.gpsimd
            dq.dma_start(out=out[row:row + 128, :], in_=ob)

        # prologue: xt + the chunks the first iteration's qc0 needs (the
        # loop body rebuilds chunk 0/1 during qc3 for the NEXT iteration,
        # so each iteration opens straight into (0,0) logits)
        for sc in range(NQ4):
            for dc in range(DC):
                nc.sync.dma_start(
                    out=xt_sb[dc][sc],
                    in_=xt[dc * 128:(dc + 1) * 128,
                           sc * 512:(sc + 1) * 512])
        for sc in range(NQ4):
            chunk_group(0, 0, sc)
            chunk_group(0, 1, sc)
            chunk_group(1, 0, sc)
            chunk_group(1, 1, sc)
        for sc in range(3):
            chunk_group(2, 0, sc)

        # fill-work units keyed (pair_index, kc); pair_index = 3*qc + hp.
        # Placement exploits per-slice read windows: qt chunk cc's sc-slice
        # is read only by pair (sc, cc); kt cc by all (q, cc) pairs through
        # (3, cc); V[kc] by every pair's PV. Everything except kt2 (and
        # qt2-sc3) is rebuilt for the NEXT loop iteration right after its
        # last reader, which spreads the projection work across all twelve
        # pair slots and lets each iteration open straight into logits.
        fill = {}

        def put(p, kc, fn, *args):
            fill.setdefault((p, kc), []).append((fn, args))

        # V[kc] must exist before PV(0,0,kc) fires at slot kc+LAG0 (pair 0
        # runs a deeper PV lag to widen these build windows).
        for kc in range(NK):
            if kc < 10:
                put(0, kc + 2, v_group, kc)
            else:
                put(1, kc + 6 - 16, v_group, kc)
        # kt2 window: (3,2)prev -> the (0,2) logits that read each slice
        put(1, 23 - 16, chunk_group, 2, 1, 0)
        put(1, 25 - 16, chunk_group, 2, 1, 1)
        put(2, 1, chunk_group, 2, 1, 2)
        put(2, 3, chunk_group, 2, 1, 3)
        put(3, 7, chunk_group, 2, 0, 3)    # qt2-sc3 (after prev (3,2))
        put(4, 7, chunk_group, 0, 0, 0)    # qt0-sc0 (after (0,0))
        put(5, 7, chunk_group, 1, 0, 0)    # qt1-sc0 (after (0,1))
        put(6, 7, chunk_group, 0, 0, 1)    # qt0-sc1 (after (1,0))
        put(7, 7, chunk_group, 1, 0, 1)    # qt1-sc1 (after (1,1))
        put(8, 7, chunk_group, 2, 0, 0)    # qt2-sc0 (after (0,2))
        put(9, 3, chunk_group, 0, 0, 2)    # qt0-sc2 (after (2,0))
        put(9, 9, chunk_group, 1, 0, 2)    # qt1-sc2 (after (2,1))
        put(9, 13, chunk_group, 2, 0, 1)   # qt2-sc1 (after (1,2))
        put(9, 6, chunk_group, 2, 0, 2)    # qt2-sc2 (after (2,2))
        for sc in range(4):                # kt0 after (3,0)
            put(10, 1 + 4 * sc, chunk_group, 0, 1, sc)
        put(10, 7, chunk_group, 0, 0, 3)   # qt0-sc3 (after (3,0))
        put(11, 3, chunk_group, 1, 0, 3)   # qt1-sc3 (after (3,1))

        loop = tc.For_i(0, reps, 1) if reps > 1 else nullcontext()
        with loop:
            for sc in range(NQ4):
                for dc in range(DC):
                    nc.sync.dma_start(
                        out=xt_sb[dc][sc],
                        in_=xt[dc * 128:(dc + 1) * 128,
                               sc * 512:(sc + 1) * 512])

            # ---- flat slot stream: 12 pairs x 16 kc ----
            # Each slot: logits+exp FIRST (so ACT never waits mid-slot),
            # then fill work, then a cross-pair-lagged PV, then at most one
            # deferred output-projection group. The PV queue crosses pair
            # boundaries so the previous pair's drain never sits between
            # ACT and the next pair's logits in the in-order PE stream.
            ctx_tiles = {}
            wo_queue = []
            pend = []  # (cps_pair, pbs, heads, kc, norm closure)

            def emit_pv_entry(e):
                cps_pair, pbs, heads, kc, norm = e
                for i, h in enumerate(heads):
                    nc.tensor.matmul(
                        cps_pair[i],
                        lhsT=(v_sb[kc][:, h, :]),
                        rhs=(pbs[i]),
                        start=(kc == 0), stop=(kc == NK - 1),
                    )
                if kc == NK - 1:
                    norm()

            def make_norm(qc, hp, cps_pair):
                def norm():
                    # One [65, 512] bf16 copy per head releases the cps
                    # banks ~1.3us after the last PV so the next pair's PV
                    # stream never stalls on them; the denominator is then
                    # re-staged from the copy (reciprocal_approx_fast needs
                    # fp32 at base partition 0) off the critical path.
                    raws = []
                    for i in range(2):
                        raw = raw_pool.tile([HD + 1, 512], bf16, tag="raw")
                        nc.vector.tensor_copy(out=raw, in_=cps_pair[i])
                        raws.append(raw)
                    for i, h in enumerate((2 * hp, 2 * hp + 1)):
                        if SHUF_BC:
                            # den -> partitions 0 and 32, reciprocal, then
                            # one stream_shuffle (mask=[0]*32: each
                            # 32-partition quadrant reads its row 0)
                            # broadcasts rec across all 64 partitions.
                            # All-DVE: no DMA, fully dependency-tracked.
                            dd = dd_tiles[dd_ctr[0] % 4]
                            dd_ctr[0] += 1
                            nc.vector.tensor_copy(
                                out=dd[0:1, :], in_=raws[i][HD:HD + 1, :])
                            nc.vector.tensor_copy(
                                out=dd[32:33, :], in_=raws[i][HD:HD + 1, :])
                            nc.vector.reciprocal_approx_fast(
                                out=dd[0:33, :], in_=dd[0:33, :])
                            rbc = rbc_pool.tile([HD, 512], f32, tag="rbc")
                            nc.vector.stream_shuffle(out=rbc, in_=dd,
                                                     mask=[0] * 32)
                        else:
                            den = rec_pool.tile([1, 512], f32, tag="den")
                            nc.vector.tensor_copy(out=den,
                                                  in_=raws[i][HD:HD + 1, :])
                            rec = rec_pool.tile([1, 512], f32, tag="rec")
                            nc.vector.reciprocal_approx_fast(out=rec, in_=den)
                            rbc = rbc_pool.tile([HD, 512], f32, tag="rbc")
                            rd = rec_dram[qc * HL + h:qc * HL + h + 1, :]
                            nc.gpsimd.dma_start(out=rd, in_=rec)
                            nc.gpsimd.dma_start(out=rbc,
                                                in_=rd.to_broadcast([HD, 512]))
                        nc.vector.tensor_mul(
                            ctx_tiles[qc][hp][i * HD:(i + 1) * HD, :],
                            raws[i][0:HD, :], rbc)
                    if hp == 2 and parts != "noWo":
                        for qs in range(4):
                            wo_queue.append((qc, qs))
                return norm

            pairs = ([(qc, hp) for qc in range(NQ4) for hp in range(3)]
                     if parts != "A" else [])
            for p_idx, (qc, hp) in enumerate(pairs):
                if hp == 0:
                    ctx_tiles[qc] = [
                        ctx_pool.tile([128, 512], bf16, tag=f"ctxp{i}",
                                      name=f"ctx_pair{i}_{qc}")
                        for i in range(CC)]
                cps_pair = [ctx_psum.tile([HD + 1, 512], f32, tag=f"cps{i}",
                                          name=f"cps{i}_{qc}_{hp}")
                            for i in range(2)]
                norm = make_norm(qc, hp, cps_pair)
                heads = (2 * hp, 2 * hp + 1)

                def emit_logits(kc):
                    # both heads' logits into one 2-bank psum tile; with
                    # PAIR_KC two consecutive slots' K=64 matmuls run
                    # back-to-back, halving the tiled->untiled PE stream
                    # transitions (measured ~16 ns/mm on HW).
                    lg = lg_psum.tile([128, 1024], f32, tag="lg")
                    if PAD_LOGITS:
                        for i, ktp in enumerate((ktA_sb, ktB_sb)):
                            nc.tensor.matmul(
                                lg[:, i * 512:(i + 1) * 512],
                                lhsT=(ktp[hp][:, kc * 128:(kc + 1) * 128]),
                                rhs=(qt_sb[hp][:, qc * 512:(qc + 1) * 512]),
                                start=True, stop=True,
                            )
                    else:
                        for i in range(2):
                            off = i * HD
                            nc.tensor.matmul(
                                lg[:, i * 512:(i + 1) * 512],
                                lhsT=(kt_sb[hp][off:off + HD,
                                                kc * 128:(kc + 1) * 128]),
                                rhs=(qt_sb[hp][off:off + HD,
                                               qc * 512:(qc + 1) * 512]),
                                start=True, stop=True,
                            )
                    return lg

                def emit_exp(kc, lg):
                    pb = probs_pool.tile([128, 1024], bf16, tag="pb")
                    nc.scalar.activation(
                        out=pb, in_=lg, func=AF.Exp,
                        bias=maskb_sb[:, kc:kc + 1], scale=0.125,
                    )
                    pend.append((cps_pair,
                                 (pb[:, 0:512], pb[:, 512:1024]),
                                 heads, kc, norm))

                def slot_rest(kc, pops_budget):
                    if not PV_FIRST:
                        for fn, args in fill.get((p_idx, kc), ()):
                            fn(*args)
                    # early pairs run deep (V-build windows); the stream end
                    # drains the queue ahead of time to shorten the tail
                    lag = (LAG0 if p_idx < NP0 else
                           2 if (p_idx == 11 and kc >= 12) else LAG)
                    pops = 0
                    while len(pend) > lag and pops < pops_budget:
                        emit_pv_entry(pend.pop(0))
                        pops += 1
                    if PV_FIRST:
                        for fn, args in fill.get((p_idx, kc), ()):
                            fn(*args)
                    if kc % 4 == 3 and wo_queue:
                        wqc, qs = wo_queue.pop(0)
                        wo_group(ctx_tiles[wqc], wqc, qs)

                if PAIR_KC:
                    for kck in range(0, NK, 2):
                        lg0 = emit_logits(kck)
                        lg1 = emit_logits(kck + 1)
                        emit_exp(kck, lg0)
                        emit_exp(kck + 1, lg1)
                        if MERGE_REST:
                            for kc in (kck, kck + 1):
                                for fn, args in fill.get((p_idx, kc), ()):
                                    fn(*args)
                            lag = (LAG0 if p_idx < 2 else
                                   2 if (p_idx == 11 and kck >= 12) else LAG)
                            pops = 0
                            while len(pend) > lag and pops < 4:
                                emit_pv_entry(pend.pop(0))
                                pops += 1
                            if wo_queue and kck % 4 == 2:
                                wqc, qs = wo_queue.pop(0)
                                wo_group(ctx_tiles[wqc], wqc, qs)
                        else:
                            slot_rest(kck, 2)
                            slot_rest(kck + 1, 2)
                    continue
                for kc in range(NK):
                    lg = emit_logits(kc)
                    emit_exp(kc, lg)
                    slot_rest(kc, 2)

            for e in pend:
                emit_pv_entry(e)
            pend.clear()
            # kt1 rebuild in the drain: PE work spanning the loop boundary
            # keeps the HAM clock gate from re-throttling (the next
            # iteration's qc0 would otherwise run its first ~15us at
            # half clock), and it overlaps the last pair's normalization
            # chain on DVE. (kt1's window opens after (3,1).)
            if parts != "A":
                for sc in range(4):
                    chunk_group(1, 1, sc)
                    if sc >= 2 and wo_queue:
                        wqc, qs = wo_queue.pop(0)
                        wo_group(ctx_tiles[wqc], wqc, qs)
            while wo_queue:
                wqc, qs = wo_queue.pop(0)
                wo_group(ctx_tiles[wqc], wqc, qs)

    nc.compile()
    return nc


def _get_nc():
    if "nc" not in _cache:
        _cache["nc"] = _build_nc()
    return _cache["nc"]


def make_in_maps(x, mask, Wq, bq, Wk, bk, Wv, bv, Wo):
    """Per-core input maps for the SPMD kernel. Core i: batch i//2, heads i%2."""
    import ml_dtypes
    bf16 = ml_dtypes.bfloat16
    x = np.asarray(x, np.float32)
    mask = np.asarray(mask, np.float32)
    in_maps = []
    for core in range(8):
        b, g = divmod(core, 2)
        sl = slice(g * CPB, (g + 1) * CPB)
        bqk_arr = np.stack([np.asarray(bq, np.float32)[sl],
                            np.asarray(bk, np.float32)[sl]])  # [2, 384]
        in_maps.append({
            "xt": np.ascontiguousarray(x[b].T).astype(bf16),
            "wq": np.ascontiguousarray(np.asarray(Wq, np.float32)[:, sl]).astype(bf16),
            "wk": np.ascontiguousarray(np.asarray(Wk, np.float32)[:, sl]).astype(bf16),
            "wv": np.ascontiguousarray(np.asarray(Wv, np.float32)[:, sl]).astype(bf16),
            "wo": np.ascontiguousarray(np.asarray(Wo, np.float32)[sl, :]).astype(bf16),
            # [128, 2*CC]: per-partition bias columns, q then k
            "bqk": np.ascontiguousarray(
                bqk_arr.reshape(2, CC, 128).transpose(2, 0, 1).reshape(128, 2 * CC)),
            "bv": np.ascontiguousarray(np.broadcast_to(
                np.asarray(bv, np.float32)[sl], (128, CPB))).astype(bf16),
            "maskb": np.ascontiguousarray(
                (mask[b, 0, 0, :] * NEG_BIG).reshape(NK, 128).T),
        })
    return in_maps


def combine(results, bo):
    out = np.empty((4, S, D), np.float32)
    for b in range(4):
        out[b] = results[2 * b]["out"] + results[2 * b + 1]["out"] \
            + np.asarray(bo, np.float32)
    return out


def kernel(x, mask, Wq, bq, Wk, bk, Wv, bv, Wo, bo):
    from concourse.bass_utils import run_bass_kernel_spmd

    nc = _get_nc()
    in_maps = make_in_maps(x, mask, Wq, bq, Wk, bk, Wv, bv, Wo)
    res = run_bass_kernel_spmd(nc, in_maps, list(range(8))).results
    return combine(res, bo)



# revision 43
# speedup vs baseline: 1.0097x; 1.0097x over previous
"""Multi-head attention (B=4, S=2048, D=768, H=12) on 8 TRN2 NeuronCores.

Sharding: core i handles batch b = i//2 and head-group g = i%2 (6 heads of 64).
Each core computes Q/K/V projections for its head slice, attention, and a
partial output projection (row-slice of Wo). Host sums the two partials per
batch and adds bo.

Device layout choices:
  - x is fed pre-transposed as xT [D, S] so all projection matmuls contract
    over D on the partition dim.
  - Q, K are produced transposed: QT/KT [384, S] (head dim on partitions).
  - logits are computed transposed, logitsT [k, q]: lhsT = KT_h [64, k-tile],
    rhs = QT_h [64, q-tile]. The additive mask (per-k) then lands on the
    partition dim, so it rides the exp() activation's per-partition bias.
    Two consecutive slots' logits (4 K=64 mms) are emitted back-to-back
    (PAIR_KC): longer uniform K=64 runs lower the per-mm cost vs
    alternating with full-array mms (HW-measured ~16 ns/mm).
  - Softmax skips max-subtraction (logits are O(5), exp is safe in fp32);
    masked positions get bias -1e9 -> exp == 0.
  - V is kept in natural [k, c] layout, augmented with a ones column, so the
    PV matmul (lhsT = V'_h [k-tile, 65], rhs = probsT [k-tile, q-tile])
    accumulates both ctxT [64, q] and the softmax denominator (row 64) in one
    accumulation group.
  - Normalization: the denominator row is copied to partitions 0 and 32
    of a static once-memset staging tile, reciprocal_approx_fast'd in
    place ([0:33]: custom-DVE ops need base partition 0, and every lane
    must see defined fp32 data — garbage lanes NaN'd whole first-exec
    outputs), then broadcast across all 64 partitions with one DVE
    stream_shuffle (mask=[0]*32 replicates each 32-partition quadrant's
    row 0) — all-DVE, no DMA (SHUF_BC); the normalized pair lands
    stacked in one [128, 512] tile.
  - Output projection contracts over the stacked head-pair dim: lhsT =
    ctx_pair [128, q-tile], rhs = Wo_pair [128, e-tile], K=128 full array,
    accumulating 3 pairs into one PSUM tile; result is in natural [q, e]
    layout for the store.
  - Scheduling: the softmax exp stream on the scalar engine is the binding
    resource (~200us/core: 6 heads x 2048^2 logits / 128 lanes / 1.2GHz,
    exp runs nowhere else), so the whole kernel is a flat stream of 192
    (pair, kc) slots paced by one [128, 1024] exp per slot. Each slot emits
    logits+exp first, then one fill-work unit, then a cross-pair-lagged PV,
    then at most one deferred output-projection group. The fill map spreads
    the V builds and QT/KT projection chunks across the stream using their
    exact read windows (qt chunk cc's sc-slice is read only by pair
    (sc, cc); kt cc is locked until (3, cc)), rebuilding most chunks for
    the NEXT loop iteration right after their last reader so an iteration
    opens straight into logits. kt1 rebuilds in the drain bridge the loop
    boundary so the PE HAM clock gate stays at full rate.
  - The softmax normalization chain (raw-copy -> reciprocal ->
    stream_shuffle broadcast -> multiply) runs off the critical path: a
    single [65, 512] bf16 copy per head releases the PSUM accumulators
    ~1.3us after the last PV. The earlier rec->dram->readback DMA pair was
    an untracked-DRAM ordering hazard: emission-order changes raced it and
    corrupted single-shot runs while reps loops masked the corruption
    (identical inputs -> stale reads look correct). SHUF_BC removes the
    hazard class entirely, which is what makes PAIR_KC and HW_DMA safe.
  - V bias rides a DVE broadcast add (VB_DVE) instead of a rank-1 matmul,
    keeping the V build to 6 full-K matmuls (measured -8us).
  - All matmul operands are bf16 (full PE speed; fp32 PSUM accumulate).
  - Measured on HW (8-core SPMD, per-iteration of a reps loop):
    baseline 302us -> 282-285us depending on ambient device state (the
    device's effective clock drifts ~5% between sessions, especially
    after NRT faults; judge A/B tests back-to-back only). Engine budget:
    PE ~255us busy (bottleneck), ACT exp stream ~209us (192 x ~1090ns),
    DVE ~129us, Pool/gpsimd near idle. Single-shot (reps=1) correctness
    re-verified 3x+ after every emission-order change. LAG=7 hard-faults
    the device (NRT_EXEC_UNIT_UNRECOVERABLE, 2/2 runs — suspected PE
    wait-queue overflow from too many simultaneously-blocked PVs);
    LAG=6/LAG0=7 is the deepest safe point and the LAG0=7 retune is
    worth ~2us.
"""

import numpy as np
from contextlib import ExitStack

S = 2048
D = 768
HL = 6  # heads per core
HD = 64
CPB = 384  # channels per core = HL * HD
DC = D // 128  # 6 contraction chunks
CC = CPB // 128  # 3 chunks of QT/KT partitions
NQ4 = S // 512  # 4 q chunks of 512
NK = S // 128  # 16 k chunks of 128
NEG_BIG = -1.0e9
LAG = 6   # PV trails logits by this many slots (crossing pair boundaries)
LAG0 = 7  # deeper lag on the first NP0 pairs, widening the V-build windows
          # (re-tuned +1 after PAIR_KC shifted fills ~1 slot later; -2us)
NP0 = 2   # pairs running at LAG0 before dropping to LAG
PAD_LOGITS = False  # full-K=128 zero-padded logits mms: measured +18us vs
                    # the row-tiled K=64 pairs in-context — keep False
VB_DVE = True       # V bias via DVE broadcast add (vs rank-1 ones matmul)
PAIR_KC = True      # emit two consecutive slots' logits back-to-back
                    # (safe only with SHUF_BC: with the DMA-based rec
                    # broadcast, emission-order shifts raced the rec_dram
                    # readback and corrupted single-shot runs)
SHUF_BC = True      # softmax-reciprocal broadcast via DVE stream_shuffle
                    # (den copied to partitions 0+32, mask=[0]*32) instead
                    # of the rec->dram->broadcast DMA pair: removes the only
                    # untracked-DRAM ordering hazard and 48 SWDGE DMAs
PV_FIRST = False    # within a slot: PV pops before fill work
HW_DMA = False      # out stores on HWDGE (sync) queues. Re-tried after the
                    # uninit-dd root cause was fixed: correct 4/4 fresh-process
                    # but no longer faster (285.4 vs 285.1us back-to-back) —
                    # its earlier -2us came from relieving SWDGE congestion
                    # that SHUF_BC has since eliminated. Keep the longer
                    # track record (stores on the strictly-FIFO SWDGE queue)
MERGE_REST = False  # merge both paired slots' fills+pops into one run

_cache = {}


def _build_nc(reps=1, parts="all"):
    import concourse.bass as bass
    import concourse.mybir as mybir
    import concourse.tile as tile
    from concourse import bacc
    from contextlib import nullcontext

    f32 = mybir.dt.float32
    bf16 = mybir.dt.bfloat16
    AF = mybir.ActivationFunctionType

    nc = bacc.Bacc("TRN2", target_bir_lowering=False, debug=False,
                   enable_asserts=False)

    xt = nc.dram_tensor("xt", [D, S], bf16, kind="ExternalInput").ap()
    wq = nc.dram_tensor("wq", [D, CPB], bf16, kind="ExternalInput").ap()
    wk = nc.dram_tensor("wk", [D, CPB], bf16, kind="ExternalInput").ap()
    wv = nc.dram_tensor("wv", [D, CPB], bf16, kind="ExternalInput").ap()
    wo = nc.dram_tensor("wo", [CPB, D], bf16, kind="ExternalInput").ap()
    bqk = nc.dram_tensor("bqk", [128, 2 * CC], f32, kind="ExternalInput").ap()
    bv = nc.dram_tensor("bv", [128, CPB], bf16, kind="ExternalInput").ap()
    maskb = nc.dram_tensor("maskb", [128, NK], f32, kind="ExternalInput").ap()
    out = nc.dram_tensor("out", [S, D], f32, kind="ExternalOutput").ap()
    rec_dram = nc.dram_tensor("rec_dram", [NQ4 * HL, 512], f32).ap()

    with tile.TileContext(nc) as tc, ExitStack() as top:
        const = top.enter_context(tc.tile_pool(name="const", bufs=1))

        # ---- constant loads ----
        wq_sb = const.tile([128, DC, CPB], bf16, tag="wq")
        wk_sb = const.tile([128, DC, CPB], bf16, tag="wk")
        wv_sb = const.tile([128, DC, CPB], bf16, tag="wv")
        for dc in range(DC):
            nc.sync.dma_start(out=wq_sb[:, dc, :], in_=wq[dc * 128:(dc + 1) * 128, :])
            nc.sync.dma_start(out=wk_sb[:, dc, :], in_=wk[dc * 128:(dc + 1) * 128, :])
            nc.sync.dma_start(out=wv_sb[:, dc, :], in_=wv[dc * 128:(dc + 1) * 128, :])
        wo_sb = [const.tile([128, D], bf16, tag=f"wo{p}", name=f"wo_sb{p}")
                 for p in range(CC)]
        for p in range(CC):
            nc.sync.dma_start(out=wo_sb[p], in_=wo[p * 128:(p + 1) * 128, :])
        bqk_sb = const.tile([128, 2 * CC], f32, tag="bqk")
        nc.sync.dma_start(out=bqk_sb, in_=bqk)
        bv_bc = const.tile([128, CPB], bf16, tag="bvbc")
        nc.sync.dma_start(out=bv_bc, in_=bv)
        maskb_sb = const.tile([128, NK], f32, tag="maskb")
        nc.sync.dma_start(out=maskb_sb, in_=maskb)
        if not VB_DVE:
            ones_sb = const.tile([1, 128], bf16, tag="ones")
            nc.vector.memset(ones_sb, 1.0)

        qt_sb = [const.tile([128, S], bf16, tag=f"qt{c}", name=f"qt_sb{c}") for c in range(CC)]
        # Padded-K logits: per pair, two [128, S] KT tiles. ktA holds the even
        # head's 64 KT rows in partitions 0-63 (64-127 stay zero); ktB holds
        # the odd head's in partitions 64-127 (0-63 zero). The logits matmul
        # is then a full K=128 untiled matmul (background-weight-buffer
        # LDWEIGHTS pull-ahead, FWL) with the full stacked qt as rhs — the
        # zero rows cancel the other head's contribution. Measured on HW:
        # untiled K=128 mm ~194 ns vs the old row-tiled K=64 pair ~266 ns
        # each (no concurrency, un-hidden LDWEIGHTS).
        if PAD_LOGITS:
            ktA_sb = [const.tile([128, S], bf16, tag=f"ktA{c}",
                                 name=f"ktA_sb{c}") for c in range(CC)]
            ktB_sb = [const.tile([128, S], bf16, tag=f"ktB{c}",
                                 name=f"ktB_sb{c}") for c in range(CC)]
            for c in range(CC):
                nc.vector.memset(ktA_sb[c][64:128, :], 0.0)
                nc.vector.memset(ktB_sb[c][0:64, :], 0.0)
        else:
            kt_sb = [const.tile([128, S], bf16, tag=f"kt{c}",
                                name=f"kt_sb{c}") for c in range(CC)]
        v_sb = [const.tile([128, HL, HD + 1], bf16, tag=f"v{k}", name=f"v_sb{k}") for k in range(NK)]
        # the ones column (softmax-denominator row of the PV matmul) never
        # changes: write it once here instead of per v_group per iteration
        # (16 fewer DVE memsets + sem waits per iteration)
        for k in range(NK):
            nc.vector.memset(v_sb[k][:, :, HD:HD + 1], 1.0)
        if SHUF_BC:
            # static dd staging tiles for the reciprocal-broadcast chain,
            # rows 1-31/33-63 memset once so the stream_shuffle never reads
            # uninitialized SBUF (first-exec NaN risk; later execs would be
            # silently masked by the previous run's residue)
            dd_tiles = [const.tile([HD, 512], f32, tag=f"dd{j}",
                                   name=f"dd_{j}") for j in range(4)]
            for j in range(4):
                nc.vector.memset(dd_tiles[j], 1.0)
            dd_ctr = [0]

        # xt tiles live in the never-closed const pool: reusing their SBUF
        # space would give later tile writers WAR/WAW waits on all 8 DMA
        # queues, exceeding HW sync-wait slots.
        xt_sb = [[const.tile([128, 512], bf16, tag=f"xt{dc}_{sc}",
                             name=f"xt_sb{dc}_{sc}") for sc in range(NQ4)]
                 for dc in range(DC)]

        # PSUM budget (8 banks): lg 2x2 + cps 2 + ops/mm shared 2 = 8
        lg_psum = top.enter_context(tc.tile_pool(name="lg", bufs=2, space="PSUM"))
        ctx_psum = top.enter_context(tc.tile_pool(name="cps", bufs=1, space="PSUM"))
        out_psum = top.enter_context(tc.tile_pool(name="ops", bufs=2, space="PSUM"))
        probs_pool = top.enter_context(tc.tile_pool(name="probs", bufs=12))
        rec_pool = top.enter_context(tc.tile_pool(name="rec", bufs=6))
        rbc_pool = top.enter_context(tc.tile_pool(name="rbc", bufs=4))
        raw_pool = top.enter_context(tc.tile_pool(name="raw", bufs=4))
        ctx_pool = top.enter_context(tc.tile_pool(name="ctx", bufs=2))
        outsb_pool = top.enter_context(tc.tile_pool(name="outsb", bufs=4))
        mm_psum = out_psum  # projection accumulators share the ops slots

        def chunk_group(cc, iw, sc):
            """One QT/KT projection group: 6 accumulating MMs + bias add.
            KT chunks land split across the zero-padded ktA/ktB tiles."""
            w_sb = (wq_sb, wk_sb)[iw]
            ps = mm_psum.tile([128, 512], f32, tag="ops",
                              name=f"qkps_{iw}_{cc}_{sc}")
            for dc in range(DC):
                nc.tensor.matmul(
                    ps,
                    lhsT=(w_sb[:, dc, cc * 128:(cc + 1) * 128]),
                    rhs=(xt_sb[dc][sc]),
                    start=(dc == 0), stop=(dc == DC - 1),
                )
            if iw == 0:
                nc.vector.tensor_scalar_add(
                    out=qt_sb[cc][:, sc * 512:(sc + 1) * 512], in0=ps,
                    scalar1=bqk_sb[:, cc:cc + 1],
                )
            elif PAD_LOGITS:
                sl = slice(sc * 512, (sc + 1) * 512)
                nc.vector.tensor_scalar_add(
                    out=ktA_sb[cc][0:64, sl], in0=ps[0:64, :],
                    scalar1=bqk_sb[0:64, CC + cc:CC + cc + 1],
                )
                nc.vector.tensor_scalar_add(
                    out=ktB_sb[cc][64:128, sl], in0=ps[64:128, :],
                    scalar1=bqk_sb[64:128, CC + cc:CC + cc + 1],
                )
            else:
                nc.vector.tensor_scalar_add(
                    out=kt_sb[cc][:, sc * 512:(sc + 1) * 512], in0=ps,
                    scalar1=bqk_sb[:, CC + cc:CC + cc + 1],
                )

        def v_group(kc):
            """V chunk kc, all 6 heads: natural [k, c] layout + ones column.
            bv is added on the DVE (broadcast tensor_tensor) so the PE group
            stays 6 full-K matmuls."""
            ps = mm_psum.tile([128, CPB], f32, tag="ops",
                              padded_shape=[128, 512], name=f"vps_{kc}")
            for dc in range(DC):
                nc.tensor.matmul(
                    ps,
                    lhsT=(xt_sb[dc][kc // 4][:, (kc % 4) * 128:
                                             (kc % 4 + 1) * 128]),
                    rhs=(wv_sb[:, dc, :]),
                    start=(dc == 0), stop=(VB_DVE and dc == DC - 1),
                )
            if VB_DVE:
                nc.vector.tensor_add(
                    v_sb[kc][:, :, 0:HD],
                    ps.rearrange("p (h d) -> p h d", h=HL),
                    bv_bc.rearrange("p (h d) -> p h d", h=HL),
                )
            else:
                nc.tensor.matmul(ps, lhsT=(ones_sb), rhs=(bv_bc[0:1, :]),
                                 start=False, stop=True)
                nc.vector.tensor_copy(
                    out=v_sb[kc][:, :, 0:HD],
                    in_=ps.rearrange("p (h d) -> p h d", h=HL),
                )

        def wo_group(ctx_list, wqc, qs):
            """Output projection for q-subtile qs of q-chunk wqc: 3 K=128
            matmuls (stacked head pairs) accumulating into one PSUM tile."""
            ob = outsb_pool.tile([128, D], f32, tag="ob",
                                 name=f"ob_{wqc}_{qs}")
            for e0, en in ((0, 512), (512, 256)):
                ps = out_psum.tile([128, 512], f32, tag="ops",
                                   name=f"wops_{wqc}_{qs}_{e0}")
                for p in range(CC):
                    nc.tensor.matmul(
                        ps[:, 0:en],
                        lhsT=(ctx_list[p][:, qs * 128:(qs + 1) * 128]),
                        rhs=(wo_sb[p][:, e0:e0 + en]),
                        start=(p == 0), stop=(p == CC - 1),
                    )
                nc.vector.tensor_copy(out=ob[:, e0:e0 + en],
                                      in_=ps[:, 0:en])
            row = (wqc * 4 + qs) * 128
            dq = nc.sync if HW_DMA else nc.gpsimd
            dq.dma_start(out=out[row:row + 128, :], in_=ob)

        # prologue: xt + the chunks the first iteration's qc0 needs (the
        # loop body rebuilds chunk 0/1 during qc3 for the NEXT iteration,
        # so each iteration opens straight into (0,0) logits)
        for sc in range(NQ4):
            for dc in range(DC):
                nc.sync.dma_start(
                    out=xt_sb[dc][sc],
                    in_=xt[dc * 128:(dc + 1) * 128,
                           sc * 512:(sc + 1) * 512])
        for sc in range(NQ4):
            chunk_group(0, 0, sc)
            chunk_group(0, 1, sc)
            chunk_group(1, 0, sc)
            chunk_group(1, 1, sc)
        for sc in range(3):
            chunk_group(2, 0, sc)

        # fill-work units keyed (pair_index, kc); pair_index = 3*qc + hp.
        # Placement exploits per-slice read windows: qt chunk cc's sc-slice
        # is read only by pair (sc, cc); kt cc by all (q, cc) pairs through
        # (3, cc); V[kc] by every pair's PV. Everything except kt2 (and
        # qt2-sc3) is rebuilt for the NEXT loop iteration right after its
        # last reader, which spreads the projection work across all twelve
        # pair slots and lets each iteration open straight into logits.
        fill = {}

        def put(p, kc, fn, *args):
            fill.setdefault((p, kc), []).append((fn, args))

        # V[kc] must exist before PV(0,0,kc) fires at slot kc+LAG0 (pair 0
        # runs a deeper PV lag to widen these build windows).
        for kc in range(NK):
            if kc < 10:
                put(0, kc + 2, v_group, kc)
            else:
                put(1, kc + 6 - 16, v_group, kc)
        # kt2 window: (3,2)prev -> the (0,2) logits that read each slice
        put(1, 23 - 16, chunk_group, 2, 1, 0)
        put(1, 25 - 16, chunk_group, 2, 1, 1)
        put(2, 1, chunk_group, 2, 1, 2)
        put(2, 3, chunk_group, 2, 1, 3)
        put(3, 7, chunk_group, 2, 0, 3)    # qt2-sc3 (after prev (3,2))
        put(4, 7, chunk_group, 0, 0, 0)    # qt0-sc0 (after (0,0))
        put(5, 7, chunk_group, 1, 0, 0)    # qt1-sc0 (after (0,1))
        put(6, 7, chunk_group, 0, 0, 1)    # qt0-sc1 (after (1,0))
        put(7, 7, chunk_group, 1, 0, 1)    # qt1-sc1 (after (1,1))
        put(8, 7, chunk_group, 2, 0, 0)    # qt2-sc0 (after (0,2))
        put(9, 3, chunk_group, 0, 0, 2)    # qt0-sc2 (after (2,0))
        put(9, 9, chunk_group, 1, 0, 2)    # qt1-sc2 (after (2,1))
        put(9, 13, chunk_group, 2, 0, 1)   # qt2-sc1 (after (1,2))
        put(9, 6, chunk_group, 2, 0, 2)    # qt2-sc2 (after (2,2))
        for sc in range(4):                # kt0 after (3,0)
            put(10, 1 + 4 * sc, chunk_group, 0, 1, sc)
        put(10, 7, chunk_group, 0, 0, 3)   # qt0-sc3 (after (3,0))
        put(11, 3, chunk_group, 1, 0, 3)   # qt1-sc3 (after (3,1))

        loop = tc.For_i(0, reps, 1) if reps > 1 else nullcontext()
        with loop:
            for sc in range(NQ4):
                for dc in range(DC):
                    nc.sync.dma_start(
                        out=xt_sb[dc][sc],
                        in_=xt[dc * 128:(dc + 1) * 128,
                               sc * 512:(sc + 1) * 512])

            # ---- flat slot stream: 12 pairs x 16 kc ----
            # Each slot: logits+exp FIRST (so ACT never waits mid-slot),
            # then fill work, then a cross-pair-lagged PV, then at most one
            # deferred output-projection group. The PV queue crosses pair
            # boundaries so the previous pair's drain never sits between
            # ACT and the next pair's logits in the in-order PE stream.
            ctx_tiles = {}
            wo_queue = []
            pend = []  # (cps_pair, pbs, heads, kc, norm closure)

            def emit_pv_entry(e):
                cps_pair, pbs, heads, kc, norm = e
                for i, h in enumerate(heads):
                    nc.tensor.matmul(
                        cps_pair[i],
                        lhsT=(v_sb[kc][:, h, :]),
                        rhs=(pbs[i]),
                        start=(kc == 0), stop=(kc == NK - 1),
                    )
                if kc == NK - 1:
                    norm()

            def make_norm(qc, hp, cps_pair):
                def norm():
                    # One [65, 512] bf16 copy per head releases the cps
                    # banks ~1.3us after the last PV so the next pair's PV
                    # stream never stalls on them; the denominator is then
                    # re-staged from the copy (reciprocal_approx_fast needs
                    # fp32 at base partition 0) off the critical path.
                    raws = []
                    for i in range(2):
                        raw = raw_pool.tile([HD + 1, 512], bf16, tag="raw")
                        nc.vector.tensor_copy(out=raw, in_=cps_pair[i])
                        raws.append(raw)
                    for i, h in enumerate((2 * hp, 2 * hp + 1)):
                        if SHUF_BC:
                            # den -> partitions 0 and 32, reciprocal, then
                            # one stream_shuffle (mask=[0]*32: each
                            # 32-partition quadrant reads its row 0)
                            # broadcasts rec across all 64 partitions.
                            # All-DVE: no DMA, fully dependency-tracked.
                            dd = dd_tiles[dd_ctr[0] % 4]
                            dd_ctr[0] += 1
                            nc.vector.tensor_copy(
                                out=dd[0:1, :], in_=raws[i][HD:HD + 1, :])
                            nc.vector.tensor_copy(
                                out=dd[32:33, :], in_=raws[i][HD:HD + 1, :])
                            nc.vector.reciprocal_approx_fast(
                                out=dd[0:33, :], in_=dd[0:33, :])
                            rbc = rbc_pool.tile([HD, 512], f32, tag="rbc")
                            nc.vector.stream_shuffle(out=rbc, in_=dd,
                                                     mask=[0] * 32)
                        else:
                            den = rec_pool.tile([1, 512], f32, tag="den")
                            nc.vector.tensor_copy(out=den,
                                                  in_=raws[i][HD:HD + 1, :])
                            rec = rec_pool.tile([1, 512], f32, tag="rec")
                            nc.vector.reciprocal_approx_fast(out=rec, in_=den)
                            rbc = rbc_pool.tile([HD, 512], f32, tag="rbc")
                            rd = rec_dram[qc * HL + h:qc * HL + h + 1, :]
                            nc.gpsimd.dma_start(out=rd, in_=rec)
                            nc.gpsimd.dma_start(out=rbc,
                                                in_=rd.to_broadcast([HD, 512]))
                        nc.vector.tensor_mul(
                            ctx_tiles[qc][hp][i * HD:(i + 1) * HD, :],
                            raws[i][0:HD, :], rbc)
                    if hp == 2 and parts != "noWo":
                        for qs in range(4):
                            wo_queue.append((qc, qs))
                return norm

            pairs = ([(qc, hp) for qc in range(NQ4) for hp in range(3)]
                     if parts != "A" else [])
            for p_idx, (qc, hp) in enumerate(pairs):
                if hp == 0:
                    ctx_tiles[qc] = [
                        ctx_pool.tile([128, 512], bf16, tag=f"ctxp{i}",
                                      name=f"ctx_pair{i}_{qc}")
                        for i in range(CC)]
                cps_pair = [ctx_psum.tile([HD + 1, 512], f32, tag=f"cps{i}",
                                          name=f"cps{i}_{qc}_{hp}")
                            for i in range(2)]
                norm = make_norm(qc, hp, cps_pair)
                heads = (2 * hp, 2 * hp + 1)

                def emit_logits(kc):
                    # both heads' logits into one 2-bank psum tile; with
                    # PAIR_KC two consecutive slots' K=64 matmuls run
                    # back-to-back, halving the tiled->untiled PE stream
                    # transitions (measured ~16 ns/mm on HW).
                    lg = lg_psum.tile([128, 1024], f32, tag="lg")
                    if PAD_LOGITS:
                        for i, ktp in enumerate((ktA_sb, ktB_sb)):
                            nc.tensor.matmul(
                                lg[:, i * 512:(i + 1) * 512],
                                lhsT=(ktp[hp][:, kc * 128:(kc + 1) * 128]),
                                rhs=(qt_sb[hp][:, qc * 512:(qc + 1) * 512]),
                                start=True, stop=True,
                            )
                    else:
                        for i in range(2):
                            off = i * HD
                            nc.tensor.matmul(
                                lg[:, i * 512:(i + 1) * 512],
                                lhsT=(kt_sb[hp][off:off + HD,
                                                kc * 128:(kc + 1) * 128]),
                                rhs=(qt_sb[hp][off:off + HD,
                                               qc * 512:(qc + 1) * 512]),
                                start=True, stop=True,
                            )
                    return lg

                def emit_exp(kc, lg):
                    pb = probs_pool.tile([128, 1024], bf16, tag="pb")
                    nc.scalar.activation(
                        out=pb, in_=lg, func=AF.Exp,
                        bias=maskb_sb[:, kc:kc + 1], scale=0.125,
                    )
                    pend.append((cps_pair,
                                 (pb[:, 0:512], pb[:, 512:1024]),
                                 heads, kc, norm))

                def slot_rest(kc, pops_budget):
                    if not PV_FIRST:
                        for fn, args in fill.get((p_idx, kc), ()):
                            fn(*args)
                    # early pairs run deep (V-build windows); the stream end
                    # drains the queue ahead of time to shorten the tail
                    lag = (LAG0 if p_idx < NP0 else
                           2 if (p_idx == 11 and kc >= 12) else LAG)
                    pops = 0
                    while len(pend) > lag and pops < pops_budget:
                        emit_pv_entry(pend.pop(0))
                        pops += 1
                    if PV_FIRST:
                        for fn, args in fill.get((p_idx, kc), ()):
                            fn(*args)
                    if kc % 4 == 3 and wo_queue:
                        wqc, qs = wo_queue.pop(0)
                        wo_group(ctx_tiles[wqc], wqc, qs)

                if PAIR_KC:
                    for kck in range(0, NK, 2):
                        lg0 = emit_logits(kck)
                        lg1 = emit_logits(kck + 1)
                        emit_exp(kck, lg0)
                        emit_exp(kck + 1, lg1)
                        if MERGE_REST:
                            for kc in (kck, kck + 1):
                                for fn, args in fill.get((p_idx, kc), ()):
                                    fn(*args)
                            lag = (LAG0 if p_idx < 2 else
                                   2 if (p_idx == 11 and kck >= 12) else LAG)
                            pops = 0
                            while len(pend) > lag and pops < 4:
                                emit_pv_entry(pend.pop(0))
                                pops += 1
                            if wo_queue and kck % 4 == 2:
                                wqc, qs = wo_queue.pop(0)
                                wo_group(ctx_tiles[wqc], wqc, qs)
                        else:
                            slot_rest(kck, 2)
                            slot_rest(kck + 1, 2)
                    continue
                for kc in range(NK):
                    lg = emit_logits(kc)
                    emit_exp(kc, lg)
                    slot_rest(kc, 2)

            for e in pend:
                emit_pv_entry(e)
            pend.clear()
            # kt1 rebuild in the drain: PE work spanning the loop boundary
            # keeps the HAM clock gate from re-throttling (the next
            # iteration's qc0 would otherwise run its first ~15us at
            # half clock), and it overlaps the last pair's normalization
            # chain on DVE. (kt1's window opens after (3,1).)
            if parts != "A":
                for sc in range(4):
                    chunk_group(1, 1, sc)
                    if sc >= 2 and wo_queue:
                        wqc, qs = wo_queue.pop(0)
                        wo_group(ctx_tiles[wqc], wqc, qs)
            while wo_queue:
                wqc, qs = wo_queue.pop(0)
                wo_group(ctx_tiles[wqc], wqc, qs)

    nc.compile()
    return nc


def _get_nc():
    if "nc" not in _cache:
        _cache["nc"] = _build_nc()
    return _cache["nc"]


def make_in_maps(x, mask, Wq, bq, Wk, bk, Wv, bv, Wo):
    """Per-core input maps for the SPMD kernel. Core i: batch i//2, heads i%2."""
    import ml_dtypes
    bf16 = ml_dtypes.bfloat16
    x = np.asarray(x, np.float32)
    mask = np.asarray(mask, np.float32)
    in_maps = []
    for core in range(8):
        b, g = divmod(core, 2)
        sl = slice(g * CPB, (g + 1) * CPB)
        bqk_arr = np.stack([np.asarray(bq, np.float32)[sl],
                            np.asarray(bk, np.float32)[sl]])  # [2, 384]
        in_maps.append({
            "xt": np.ascontiguousarray(x[b].T).astype(bf16),
            "wq": np.ascontiguousarray(np.asarray(Wq, np.float32)[:, sl]).astype(bf16),
            "wk": np.ascontiguousarray(np.asarray(Wk, np.float32)[:, sl]).astype(bf16),
            "wv": np.ascontiguousarray(np.asarray(Wv, np.float32)[:, sl]).astype(bf16),
            "wo": np.ascontiguousarray(np.asarray(Wo, np.float32)[sl, :]).astype(bf16),
            # [128, 2*CC]: per-partition bias columns, q then k
            "bqk": np.ascontiguousarray(
                bqk_arr.reshape(2, CC, 128).transpose(2, 0, 1).reshape(128, 2 * CC)),
            "bv": np.ascontiguousarray(np.broadcast_to(
                np.asarray(bv, np.float32)[sl], (128, CPB))).astype(bf16),
            "maskb": np.ascontiguousarray(
                (mask[b, 0, 0, :] * NEG_BIG).reshape(NK, 128).T),
        })
    return in_maps


def combine(results, bo):
    out = np.empty((4, S, D), np.float32)
    for b in range(4):
        out[b] = results[2 * b]["out"] + results[2 * b + 1]["out"] \
            + np.asarray(bo, np.float32)
    return out


def kernel(x, mask, Wq, bq, Wk, bk, Wv, bv, Wo, bo):
    from concourse.bass_utils import run_bass_kernel_spmd

    nc = _get_nc()
    in_maps = make_in_maps(x, mask, Wq, bq, Wk, bk, Wv, bv, Wo)
    res = run_bass_kernel_spmd(nc, in_maps, list(range(8))).results
    return combine(res, bo)

